# revision 44
# baseline (speedup 1.0000x reference)
"""Fused multi-head attention (B=2, T=2048, D=2048, H=16) on 8 trn2 NeuronCores.

Sharding: core c handles batch b=c//4 and heads [4g, 4g+4), g=c%4 (tensor
parallel over heads x data parallel over batch). Each core computes its
4 heads' contribution to out[b] = attn(x[b]) @ Wo^T; the host sums the 4
partials per batch.

v2: all matmul operands fp16 (PSUM stays fp32), x^T DMA'd once into
resident SBUF, weights prefetched at kernel start, v computed directly in
[token, feature] layout (x-stationary matmuls, no PE transposes), exp
batched over j-tile pairs ([128,1024] ACT calls), causal diagonal masks as
2 resident [128,2,512] pattern tiles, software-pipelined attention inner
loop sized to exactly 8 PSUM banks, fp16 output.

Device algorithm (per core, E=512 features = 4 heads):
  P1  qT/kT = (W_s) @ x^T   [E rows as 4x(dh=128), T]   (Wq pre-scaled)
      v     = x @ Wv_s^T    [T, E]
  P2  per i-chunk (512 q), head pair: S^T pair = kT_jt^T-contract @ qT
        -> exp (ACT, [128,1024]) -> *mask (diag pairs) ->
        ctx^T += v_jt^T @ P^T ; L += ones128^T @ P^T (l replicated on all
        128 partitions, same PE tile config as ctx, no broadcast needed);
        ctx^T *= recip(L).  Upper-diagonal pairs trimmed to 256 queries.
  P3  out[t, d] = sum_e ctx^T[e, t] * WoT[e, d] -> DRAM (fp16)
"""

import numpy as np

import concourse.bass as bass
import concourse.mybir as mybir
import concourse.tile as tile
from concourse import bacc
from concourse.bass_utils import run_bass_kernel_spmd

F32 = mybir.dt.float32
F16 = mybir.dt.float16
EXP = mybir.ActivationFunctionType.Exp

B, T, D, H = 2, 2048, 2048, 16
DH = D // H          # 128
E = 512              # features per core (4 heads)
HPC = 4              # heads per core
NT = T // 128        # 16 token tiles
ND = D // 128        # 16 model-dim tiles
NE = E // 128        # 4 e-tiles per core
NI = T // 512        # 4 i-chunks (query chunks)
NJ = NT              # 16 j-tiles (key tiles)
NCH = T // 1024      # 2 big token chunks for the projections

_NC_CACHE = {}

# per-(jt, ic) mask-block class: 0 = fully masked (skip), 1 = unmasked
# (skip the mask multiply), 2 = mixed (multiply by exp(mask) elementwise)
SKIP, NOMULT, MIXED = 0, 1, 2


def _build(cls_key, causal):
    cls = np.asarray(cls_key, dtype=np.int64).reshape(NJ, NI)
    nc = bacc.Bacc(None, target_bir_lowering=False, debug=False)
    # all staged operands are host-permuted to SBUF layout [128, tile, free]
    # so each one is a single large DMA
    xt = nc.declare_dram_parameter("xt", [128, ND, T], F16, isOutput=False)
    wq = nc.declare_dram_parameter("wq", [128, ND, E], F16, isOutput=False)
    wk = nc.declare_dram_parameter("wk", [128, ND, E], F16, isOutput=False)
    wv = nc.declare_dram_parameter("wv", [128, ND, E], F16, isOutput=False)
    wo = nc.declare_dram_parameter("wo", [128, NE, D], F16, isOutput=False)
    if causal:
        em2 = nc.declare_dram_parameter("em2", [128, 2, 2, 512], F16, isOutput=False)
    else:
        em = nc.declare_dram_parameter("em", [T, T], F16, isOutput=False)
    ons = nc.declare_dram_parameter("ons", [128, 128], F16, isOutput=False)
    out = nc.declare_dram_parameter("out", [T, D], F16, isOutput=True)

    with tile.TileContext(nc) as tc:
        # ---- long-lived residents --------------------------------------
        pool_res = tc.alloc_tile_pool(name="res", bufs=1)
        ctx = [pool_res.tile([128, T], F16, name=f"ctx{m}") for m in range(NE)]
        v_sb = pool_res.tile([128, NT, E], F16)
        wo_sb = pool_res.tile([128, NE, D], F16)
        ones_sb = pool_res.tile([128, 128], F16)
        scratch = pool_res.tile([1, 8], F16)
        if causal:
            em_sb = pool_res.tile([128, 2, 2, 512], F16)

        pool_qk = tc.alloc_tile_pool(name="res_qk", bufs=1)
        qT = [pool_qk.tile([128, T], F16, name=f"qT{m}") for m in range(NE)]
        kT = [pool_qk.tile([128, T], F16, name=f"kT{m}") for m in range(NE)]

        # One set of PSUM pools for every phase: P1's v-pass borrows the
        # "l" banks and q/k the "c" banks, so there is no PSUM pool
        # transition (and no pool-release barrier) at the P1->P2 seam.
        ps_s_pool = tc.alloc_tile_pool(name="pss", bufs=2, space="PSUM")
        ps_c_pool = tc.alloc_tile_pool(name="psc", bufs=3, space="PSUM")
        ps_l_pool = tc.alloc_tile_pool(name="psl", bufs=1, space="PSUM")

        pool_p1 = tc.alloc_tile_pool(name="p1", bufs=1)
        xt_sb = pool_p1.tile([128, ND, T], F16)
        wq_sb = pool_p1.tile([128, ND, E], F16)
        wk_sb = pool_p1.tile([128, ND, E], F16)
        wv_sb = pool_p1.tile([128, ND, E], F16)

        # ---- DMA schedule: wv + per-token-block x first so the v-pass can
        # start after ~2.5MB instead of gating on the full 6MB q-projection
        # working set; the rest streams underneath the v-pass compute.
        # many small triggers up front engage more DMA queues in parallel,
        # raising effective bandwidth through the startup ramp
        for e in range(8):
            nc.sync.dma_start(out=wv_sb[:, 2 * e:2 * e + 2, :],
                              in_=wv.ap()[:, 2 * e:2 * e + 2, :])
            if e < 4:
                tsl = slice(e * 128, (e + 1) * 128)
                nc.sync.dma_start(out=xt_sb[:, :, tsl], in_=xt.ap()[:, :, tsl])
        for tb in range(4, 8):
            tsl = slice(tb * 128, (tb + 1) * 128)
            nc.sync.dma_start(out=xt_sb[:, :, tsl], in_=xt.ap()[:, :, tsl])
        for c4 in range(4):
            tsl = slice(1024 + c4 * 256, 1024 + (c4 + 1) * 256)
            nc.sync.dma_start(out=xt_sb[:, :, tsl], in_=xt.ap()[:, :, tsl])
        nc.sync.dma_start(out=wq_sb, in_=wq.ap())
        nc.sync.dma_start(out=wk_sb, in_=wk.ap())
        nc.sync.dma_start(out=ones_sb, in_=ons.ap())
        if causal:
            nc.sync.dma_start(out=em_sb, in_=em2.ap())
        nc.sync.dma_start(out=wo_sb, in_=wo.ap())

        # warm the ACT exp table set before P2 needs it
        nc.scalar.activation(scratch[0:1, 0:1], wv_sb[0:1, 0, 0:1], EXP)

        scope_p1 = nc.named_scope("P1_qkv"); scope_p1.__enter__()
        # ---- P1: v (token-major) first, then q/k (feature-major) --------
        for tbg in range(NT):
            ps = ps_c_pool.tile([128, 512], F32, name="ps_v", tag="c")
            for dt in range(ND):
                nc.tensor.matmul(
                    ps, xt_sb[:, dt, tbg * 128:(tbg + 1) * 128],
                    wv_sb[:, dt, :], start=(dt == 0), stop=(dt == ND - 1))
            if tbg % 2 == 0:
                nc.vector.tensor_copy(v_sb[:, tbg, :], ps)
            else:
                nc.scalar.copy(v_sb[:, tbg, :], ps)
        for tch in range(NCH):
            for ti, (w_sb, dst) in enumerate(((wq_sb, qT), (wk_sb, kT))):
                for m in range(NE):
                    for half in range(2):
                        tsl = slice(tch * 1024 + half * 512,
                                    tch * 1024 + (half + 1) * 512)
                        ps = ps_c_pool.tile([128, 512], F32, name="ps_qk", tag="c")
                        for dt in range(ND):
                            nc.tensor.matmul(ps, w_sb[:, dt, m * 128:(m + 1) * 128],
                                             xt_sb[:, dt, tsl],
                                             start=(dt == 0), stop=(dt == ND - 1))
                        if (m + ti + half) % 2 == 0:
                            nc.scalar.copy(dst[m][:, tsl], ps)
                        else:
                            nc.vector.tensor_copy(dst[m][:, tsl], ps)
        pool_p1.release()
        scope_p1.__exit__(None, None, None)

        scope_p2 = nc.named_scope("P2_attn"); scope_p2.__enter__()
        # ---- P2: attention + fused output projection --------------------
        # The softmax row-sums are accumulated on the (otherwise idle) DVE
        # into a Z pair-plane tile, leaving the PE stream S+ctx only; the
        # freed PE slack is filled with P3 output-projection units emitted
        # at each (chunk, head) epilogue (chunk ic-1, 4 units per head).
        p_pt = tc.alloc_tile_pool(name="p2pt", bufs=6)
        p_em = tc.alloc_tile_pool(name="p2em", bufs=3)
        p_bs = tc.alloc_tile_pool(name="p2bs", bufs=3)
        p_z = tc.alloc_tile_pool(name="p2z", bufs=3)
        p_ot = tc.alloc_tile_pool(name="p2ot", bufs=6)

        def p3_unit(tt, nch):
            tsl = slice(tt * 128, (tt + 1) * 128)
            ps_o = ps_c_pool.tile([128, 512], F32, name="ps_o", tag="c")
            for et in range(NE):
                nc.tensor.matmul(
                    ps_o, ctx[et][:, tsl],
                    wo_sb[:, et, nch * 512:(nch + 1) * 512],
                    start=(et == 0), stop=(et == NE - 1))
            ot = p_ot.tile([128, 512], F16, name="ot")
            if (tt + nch) % 2 == 0:
                nc.scalar.copy(ot, ps_o)
            else:
                nc.vector.tensor_copy(ot, ps_o)
            nc.sync.dma_start(
                out=out.ap()[tsl, nch * 512:(nch + 1) * 512], in_=ot)

        for ic in range(NI):
            isl = slice(ic * 512, (ic + 1) * 512)
            surv = [jt for jt in range(NJ) if cls[jt, ic] != SKIP]
            assert surv, f"i-chunk {ic}: every key block masked"
            first, last = surv[0], surv[-1]
            pairs = [tuple(surv[i:i + 2]) for i in range(0, len(surv), 2)]
            # P3 units for the previous chunk, shared across this chunk's
            # four head loops to fill the PE's exp-wait slack
            ic_units = [(4 * (ic - 1) + tt, u)
                        for tt in range(4) for u in range(4)] if ic else []
            for h in range(HPC):
                cps = ps_c_pool.tile([128, 512], F32, name="ps_c", tag="c")
                zt = p_z.tile([128, 2, 512], F16, name="zt")
                # h0's first slot stays empty: the previous chunk's last
                # ctx normalization is still draining on the DVE
                skip_slots = 1 if h == 0 else 0
                inunits = 0

                def flush(prev):
                    ppt, ppr, pqo, pqn = prev
                    psl = slice(pqo, pqo + pqn)
                    for j, jt in enumerate(ppr):
                        st, sp = jt == first, jt == last
                        nc.tensor.matmul(
                            cps[:, psl], v_sb[:, jt, h * 128:(h + 1) * 128],
                            ppt[:, j, 0:pqn], start=st, stop=sp,
                            skip_group_check=True)

                prev = None
                for pr in pairs:
                    # query-range trim: the upper diagonal pair only attends
                    # to the last 256 queries of the chunk
                    if causal and cls[pr[0], ic] == MIXED and pr[0] == 4 * ic + 2:
                        qo, qn = 256, 256
                    else:
                        qo, qn = 0, 512
                    # mask operand (None / resident slice / DMA'd)
                    emop = None
                    if causal and cls[pr[0], ic] == MIXED:
                        # both diagonal pairs reduce to the (o0, o1) patterns
                        # over their query window
                        emop = em_sb[:, 0, :, 0:qn]
                    elif not causal and any(cls[jt, ic] == MIXED for jt in pr):
                        emt = p_em.tile([128, 2, 512], F16, name="emt")
                        for j, jt in enumerate(pr):
                            if cls[jt, ic] == MIXED:
                                nc.sync.dma_start(
                                    out=emt[:, j, :],
                                    in_=em.ap()[jt * 128:(jt + 1) * 128, isl])
                            else:
                                nc.vector.memset(emt[:, j, :], 1.0)
                        emop = emt[:, :, 0:qn]
                    ps_s = ps_s_pool.tile([128, 2, 512], F32, name="ps_s")
                    for j, jt in enumerate(pr):
                        nc.tensor.matmul(
                            ps_s[:, j, 0:qn], kT[h][:, jt * 128:(jt + 1) * 128],
                            qT[h][:, ic * 512 + qo:ic * 512 + qo + qn],
                            start=True, stop=True)
                    pt = p_pt.tile([128, 2, 512], F16, name="pt")
                    if len(pr) == 2:
                        nc.scalar.activation(
                            pt[:, :, 0:qn], ps_s[:, :, 0:qn], EXP)
                    else:
                        nc.scalar.activation(
                            pt[:, 0, 0:qn], ps_s[:, 0, 0:qn], EXP)
                    if emop is not None:
                        if len(pr) == 2:
                            nc.vector.tensor_mul(
                                pt[:, :, 0:qn], pt[:, :, 0:qn], emop)
                        else:
                            nc.vector.tensor_mul(
                                pt[:, 0, 0:qn], pt[:, 0, 0:qn], emop[:, 0, :])
                    # DVE row-sum accumulation into the Z pair-planes
                    zsl = slice(qo, qo + qn)
                    if prev is None:
                        if len(pr) == 2:
                            nc.vector.tensor_copy(zt, pt)
                        else:
                            nc.vector.tensor_copy(zt[:, 0, :], pt[:, 0, :])
                            nc.vector.memset(zt[:, 1, :], 0.0)
                    elif len(pr) == 2:
                        with nc.allow_low_precision(reason="softmax denom f16"):
                            nc.vector.tensor_add(
                                zt[:, :, zsl], zt[:, :, zsl], pt[:, :, 0:qn])
                    else:
                        with nc.allow_low_precision(reason="softmax denom f16"):
                            nc.vector.tensor_add(
                                zt[:, 0, zsl], zt[:, 0, zsl], pt[:, 0, 0:qn])
                    if prev is not None:
                        flush(prev)
                        if skip_slots:
                            skip_slots -= 1
                        elif ic_units and inunits < 2:
                            p3_unit(*ic_units.pop(0))
                            inunits += 1
                    prev = (pt, pr, qo, qn)
                flush(prev)
                lps = ps_l_pool.tile([128, 512], F32, name="ps_l", tag="l")
                for j in range(2):
                    nc.tensor.matmul(lps, ones_sb, zt[:, j, :],
                                     start=(j == 0), stop=(j == 1))
                bsb = p_bs.tile([128, 512], F32, name="bsb")
                nc.vector.reciprocal_approx_fast(out=bsb, in_=lps)
                nc.vector.tensor_mul(ctx[h][:, isl], cps, bsb)
                nflush = len(ic_units) if h == HPC - 1 else min(2, len(ic_units))
                for _ in range(nflush):
                    p3_unit(*ic_units.pop(0))
        scope_p2.__exit__(None, None, None)
        scope_p3 = nc.named_scope("P3_out"); scope_p3.__enter__()
        # ---- P3 tail: last token chunk ----------------------------------
        for tt in range(12, 16):
            for nch in range(NI):
                p3_unit(tt, nch)
        for p in (p_ot, p_z, p_bs, p_em, p_pt):
            p.release()
        for p in (ps_l_pool, ps_c_pool, ps_s_pool):
            p.release()
        pool_qk.release()
        pool_res.release()
        scope_p3.__exit__(None, None, None)

    nc.compile()
    return nc


def _get_nc(cls_key, causal):
    key = (cls_key, causal)
    if key not in _NC_CACHE:
        _NC_CACHE[key] = _build(cls_key, causal)
    return _NC_CACHE[key]


def _causal_pattern(o):
    p = np.arange(128)[:, None]
    f = np.arange(512)[None, :]
    return (p + o * 128 <= f).astype(np.float16)


def kernel(x, Wq, Wk, Wv, Wo, attn_mask):
    x = np.asarray(x, dtype=np.float32)
    Wq = np.asarray(Wq, dtype=np.float32)
    Wk = np.asarray(Wk, dtype=np.float32)
    Wv = np.asarray(Wv, dtype=np.float32)
    Wo = np.asarray(Wo, dtype=np.float32)
    mask = np.asarray(attn_mask, dtype=np.float32).reshape(T, T)

    emT = np.ascontiguousarray(np.exp(mask).T)
    scale = np.float32(1.0 / np.sqrt(DH))

    blocks = emT.reshape(NJ, 128, NI, 512)
    cls = np.full((NJ, NI), MIXED, dtype=np.int64)
    for jt in range(NJ):
        for ic in range(NI):
            sub = blocks[jt, :, ic, :]
            if not sub.any():
                cls[jt, ic] = SKIP
            elif np.all(sub == 1.0):
                cls[jt, ic] = NOMULT
    cls_key = tuple(cls.flatten().tolist())

    # causal fast path: survivors are a prefix, MIXED blocks are the last 4
    # of each i-chunk and match the canonical diagonal patterns
    causal = True
    pat = [_causal_pattern(o).astype(np.float32) for o in range(4)]
    for ic in range(NI):
        surv = [jt for jt in range(NJ) if cls[jt, ic] != SKIP]
        mix = [jt for jt in range(NJ) if cls[jt, ic] == MIXED]
        if surv != list(range(4 * ic + 4)) or mix != list(range(4 * ic, 4 * ic + 4)):
            causal = False
            break
        for jt in mix:
            if not np.array_equal(blocks[jt, :, ic, :], pat[jt - 4 * ic]):
                causal = False
                break
        if not causal:
            break

    em2 = np.zeros((128, 2, 2, 512), dtype=np.float16)
    for kk in range(2):
        for j in range(2):
            em2[:, kk, j, :] = _causal_pattern(2 * kk + j)

    def _perm(a, ntile):
        # [ntile*128, F] -> [128, ntile, F] contiguous fp16
        f = a.shape[1]
        return np.ascontiguousarray(
            a.reshape(ntile, 128, f).transpose(1, 0, 2)).astype(np.float16)

    xT = [_perm(x[b].T, ND) for b in range(B)]
    emT16 = emT.astype(np.float16)

    in_maps = []
    for c in range(8):
        b, g = c // 4, c % 4
        rows = slice(E * g, E * (g + 1))
        m = {
            "xt": xT[b],
            "wq": _perm((Wq[rows, :] * scale).T, ND),
            "wk": _perm(Wk[rows, :].T, ND),
            "wv": _perm(Wv[rows, :].T, ND),
            "wo": _perm(Wo[:, rows].T, NE),
            "ons": np.ones((128, 128), dtype=np.float16),
        }
        if causal:
            m["em2"] = em2
        else:
            m["em"] = emT16
        in_maps.append(m)

    global _LAST_IN_MAPS, _LAST_NC
    _LAST_IN_MAPS = in_maps
    nc = _get_nc(cls_key, causal)
    _LAST_NC = nc
    res = run_bass_kernel_spmd(nc, in_maps, list(range(8)))
    outs = [r["out"].astype(np.float32) for r in res.results]
    full = np.stack([
        outs[0] + outs[1] + outs[2] + outs[3],
        outs[4] + outs[5] + outs[6] + outs[7],
    ]).astype(np.float32)
    return full


# revision 46
# speedup vs baseline: 1.0051x; 1.0051x over previous
"""Fused multi-head attention (B=2, T=2048, D=2048, H=16) on 8 trn2 NeuronCores.

Sharding: core c handles batch b=c//4 and heads [4g, 4g+4), g=c%4 (tensor
parallel over heads x data parallel over batch). Each core computes its
4 heads' contribution to out[b] = attn(x[b]) @ Wo^T; the host sums the 4
partials per batch.

v2: all matmul operands fp16 (PSUM stays fp32), x^T DMA'd once into
resident SBUF, weights prefetched at kernel start, v computed directly in
[token, feature] layout (x-stationary matmuls, no PE transposes), exp
batched over j-tile pairs ([128,1024] ACT calls), causal diagonal masks as
2 resident [128,2,512] pattern tiles, software-pipelined attention inner
loop sized to exactly 8 PSUM banks, fp16 output.

Device algorithm (per core, E=512 features = 4 heads):
  P1  qT/kT = (W_s) @ x^T   [E rows as 4x(dh=128), T]   (Wq pre-scaled)
      v     = x @ Wv_s^T    [T, E]
  P2  per i-chunk (512 q), head pair: S^T pair = kT_jt^T-contract @ qT
        -> exp (ACT, [128,1024]) -> *mask (diag pairs) ->
        ctx^T += v_jt^T @ P^T ; L += ones128^T @ P^T (l replicated on all
        128 partitions, same PE tile config as ctx, no broadcast needed);
        ctx^T *= recip(L).  Upper-diagonal pairs trimmed to 256 queries.
  P3  out[t, d] = sum_e ctx^T[e, t] * WoT[e, d] -> DRAM (fp16)
"""

import numpy as np

import concourse.bass as bass
import concourse.mybir as mybir
import concourse.tile as tile
from concourse import bacc
from concourse.bass_utils import run_bass_kernel_spmd

F32 = mybir.dt.float32
F16 = mybir.dt.float16
EXP = mybir.ActivationFunctionType.Exp

B, T, D, H = 2, 2048, 2048, 16
DH = D // H          # 128
E = 512              # features per core (4 heads)
HPC = 4              # heads per core
NT = T // 128        # 16 token tiles
ND = D // 128        # 16 model-dim tiles
NE = E // 128        # 4 e-tiles per core
NI = T // 512        # 4 i-chunks (query chunks)
NJ = NT              # 16 j-tiles (key tiles)
NCH = T // 1024      # 2 big token chunks for the projections

_NC_CACHE = {}

# per-(jt, ic) mask-block class: 0 = fully masked (skip), 1 = unmasked
# (skip the mask multiply), 2 = mixed (multiply by exp(mask) elementwise)
SKIP, NOMULT, MIXED = 0, 1, 2


def _build(cls_key, causal):
    cls = np.asarray(cls_key, dtype=np.int64).reshape(NJ, NI)
    nc = bacc.Bacc(None, target_bir_lowering=False, debug=False)
    # all staged operands are host-permuted to SBUF layout [128, tile, free]
    # so each one is a single large DMA
    xt = nc.declare_dram_parameter("xt", [128, ND, T], F16, isOutput=False)
    wq = nc.declare_dram_parameter("wq", [128, ND, E], F16, isOutput=False)
    wk = nc.declare_dram_parameter("wk", [128, ND, E], F16, isOutput=False)
    wv = nc.declare_dram_parameter("wv", [128, ND, E], F16, isOutput=False)
    wo = nc.declare_dram_parameter("wo", [128, NE, D], F16, isOutput=False)
    if causal:
        em2 = nc.declare_dram_parameter("em2", [128, 2, 2, 512], F16, isOutput=False)
    else:
        em = nc.declare_dram_parameter("em", [T, T], F16, isOutput=False)
    ons = nc.declare_dram_parameter("ons", [128, 128], F16, isOutput=False)
    out = nc.declare_dram_parameter("out", [T, D], F16, isOutput=True)

    with tile.TileContext(nc) as tc:
        # ---- long-lived residents --------------------------------------
        pool_res = tc.alloc_tile_pool(name="res", bufs=1)
        ctx = [pool_res.tile([128, T], F16, name=f"ctx{m}") for m in range(NE)]
        v_sb = pool_res.tile([128, NT, E], F16)
        wo_sb = pool_res.tile([128, NE, D], F16)
        ones_sb = pool_res.tile([128, 128], F16)
        scratch = pool_res.tile([1, 8], F16)
        if causal:
            em_sb = pool_res.tile([128, 2, 2, 512], F16)

        pool_qk = tc.alloc_tile_pool(name="res_qk", bufs=1)
        qT = [pool_qk.tile([128, T], F16, name=f"qT{m}") for m in range(NE)]
        kT = [pool_qk.tile([128, T], F16, name=f"kT{m}") for m in range(NE)]

        # One set of PSUM pools for every phase: P1's v-pass borrows the
        # "l" banks and q/k the "c" banks, so there is no PSUM pool
        # transition (and no pool-release barrier) at the P1->P2 seam.
        ps_s_pool = tc.alloc_tile_pool(name="pss", bufs=2, space="PSUM")
        ps_c_pool = tc.alloc_tile_pool(name="psc", bufs=3, space="PSUM")
        ps_l_pool = tc.alloc_tile_pool(name="psl", bufs=1, space="PSUM")

        pool_p1 = tc.alloc_tile_pool(name="p1", bufs=1)
        xt_sb = pool_p1.tile([128, ND, T], F16)
        wq_sb = pool_p1.tile([128, ND, E], F16)
        wk_sb = pool_p1.tile([128, ND, E], F16)
        wv_sb = pool_p1.tile([128, ND, E], F16)

        # ---- DMA schedule: wv + per-token-block x first so the v-pass can
        # start after ~2.5MB instead of gating on the full 6MB q-projection
        # working set; the rest streams underneath the v-pass compute.
        nc.sync.dma_start(out=wv_sb[:, 0:4, :], in_=wv.ap()[:, 0:4, :])
        nc.sync.dma_start(out=xt_sb[:, :, 0:128], in_=xt.ap()[:, :, 0:128])
        for q in range(1, 4):
            nc.sync.dma_start(out=wv_sb[:, 4 * q:4 * q + 4, :],
                              in_=wv.ap()[:, 4 * q:4 * q + 4, :])
        for tb in range(1, 8):
            tsl = slice(tb * 128, (tb + 1) * 128)
            nc.sync.dma_start(out=xt_sb[:, :, tsl], in_=xt.ap()[:, :, tsl])
        nc.sync.dma_start(out=xt_sb[:, :, 1024:1536], in_=xt.ap()[:, :, 1024:1536])
        nc.sync.dma_start(out=xt_sb[:, :, 1536:2048], in_=xt.ap()[:, :, 1536:2048])
        nc.sync.dma_start(out=wq_sb, in_=wq.ap())
        nc.sync.dma_start(out=wk_sb, in_=wk.ap())
        nc.sync.dma_start(out=ones_sb, in_=ons.ap())
        if causal:
            nc.sync.dma_start(out=em_sb, in_=em2.ap())
        nc.sync.dma_start(out=wo_sb, in_=wo.ap())

        # warm the ACT exp table set before P2 needs it
        nc.scalar.activation(scratch[0:1, 0:1], wv_sb[0:1, 0, 0:1], EXP)

        scope_p1 = nc.named_scope("P1_qkv"); scope_p1.__enter__()
        # ---- P1: v (token-major) first, then q/k (feature-major) --------
        for tbg in range(NT):
            ps = ps_c_pool.tile([128, 512], F32, name="ps_v", tag="c")
            for dt in range(ND):
                nc.tensor.matmul(
                    ps, xt_sb[:, dt, tbg * 128:(tbg + 1) * 128],
                    wv_sb[:, dt, :], start=(dt == 0), stop=(dt == ND - 1))
            if tbg % 2 == 0:
                nc.vector.tensor_copy(v_sb[:, tbg, :], ps)
            else:
                nc.scalar.copy(v_sb[:, tbg, :], ps)
        for tch in range(NCH):
            for ti, (w_sb, dst) in enumerate(((wq_sb, qT), (wk_sb, kT))):
                for m in range(NE):
                    for half in range(2):
                        tsl = slice(tch * 1024 + half * 512,
                                    tch * 1024 + (half + 1) * 512)
                        ps = ps_c_pool.tile([128, 512], F32, name="ps_qk", tag="c")
                        for dt in range(ND):
                            nc.tensor.matmul(ps, w_sb[:, dt, m * 128:(m + 1) * 128],
                                             xt_sb[:, dt, tsl],
                                             start=(dt == 0), stop=(dt == ND - 1))
                        if (m + ti + half) % 2 == 0:
                            nc.scalar.copy(dst[m][:, tsl], ps)
                        else:
                            nc.vector.tensor_copy(dst[m][:, tsl], ps)
        pool_p1.release()
        scope_p1.__exit__(None, None, None)

        scope_p2 = nc.named_scope("P2_attn"); scope_p2.__enter__()
        # ---- P2: attention + fused output projection --------------------
        # The softmax row-sums are accumulated on the (otherwise idle) DVE
        # into a Z pair-plane tile, leaving the PE stream S+ctx only; the
        # freed PE slack is filled with P3 output-projection units emitted
        # at each (chunk, head) epilogue (chunk ic-1, 4 units per head).
        p_pt = tc.alloc_tile_pool(name="p2pt", bufs=6)
        p_em = tc.alloc_tile_pool(name="p2em", bufs=3)
        p_bs = tc.alloc_tile_pool(name="p2bs", bufs=3)
        p_z = tc.alloc_tile_pool(name="p2z", bufs=3)
        p_ot = tc.alloc_tile_pool(name="p2ot", bufs=6)

        def p3_unit(tt, nch, act_ok=False):
            tsl = slice(tt * 128, (tt + 1) * 128)
            ps_o = ps_c_pool.tile([128, 512], F32, name="ps_o", tag="c")
            for et in range(NE):
                nc.tensor.matmul(
                    ps_o, ctx[et][:, tsl],
                    wo_sb[:, et, nch * 512:(nch + 1) * 512],
                    start=(et == 0), stop=(et == NE - 1))
            ot = p_ot.tile([128, 512], F16, name="ot")
            # inside the attention window ACT is reserved for the exps (an
            # interleaved copy there delays the exp chain and stalls ctx)
            if act_ok and (tt + nch) % 2 == 0:
                nc.scalar.copy(ot, ps_o)
            else:
                nc.vector.tensor_copy(ot, ps_o)
            nc.sync.dma_start(
                out=out.ap()[tsl, nch * 512:(nch + 1) * 512], in_=ot)

        for ic in range(NI):
            isl = slice(ic * 512, (ic + 1) * 512)
            surv = [jt for jt in range(NJ) if cls[jt, ic] != SKIP]
            assert surv, f"i-chunk {ic}: every key block masked"
            first, last = surv[0], surv[-1]
            pairs = [tuple(surv[i:i + 2]) for i in range(0, len(surv), 2)]
            # P3 units for the previous chunk, shared across this chunk's
            # four head loops to fill the PE's exp-wait slack
            ic_units = [(4 * (ic - 1) + tt, u)
                        for tt in range(4) for u in range(4)] if ic else []
            for h in range(HPC):
                cps = ps_c_pool.tile([128, 512], F32, name="ps_c", tag="c")
                zt = p_z.tile([128, 2, 512], F16, name="zt")
                # h0's first slot stays empty: the previous chunk's last
                # ctx normalization is still draining on the DVE
                skip_slots = 1 if h == 0 else 0
                inunits = 0

                def flush(prev):
                    ppt, ppr, pqo, pqn = prev
                    psl = slice(pqo, pqo + pqn)
                    for j, jt in enumerate(ppr):
                        st, sp = jt == first, jt == last
                        nc.tensor.matmul(
                            cps[:, psl], v_sb[:, jt, h * 128:(h + 1) * 128],
                            ppt[:, j, 0:pqn], start=st, stop=sp,
                            skip_group_check=True)

                prev = None
                for pr in pairs:
                    # query-range trim: the upper diagonal pair only attends
                    # to the last 256 queries of the chunk
                    if causal and cls[pr[0], ic] == MIXED and pr[0] == 4 * ic + 2:
                        qo, qn = 256, 256
                    else:
                        qo, qn = 0, 512
                    # mask operand (None / resident slice / DMA'd)
                    emop = None
                    if causal and cls[pr[0], ic] == MIXED:
                        # both diagonal pairs reduce to the (o0, o1) patterns
                        # over their query window
                        emop = em_sb[:, 0, :, 0:qn]
                    elif not causal and any(cls[jt, ic] == MIXED for jt in pr):
                        emt = p_em.tile([128, 2, 512], F16, name="emt")
                        for j, jt in enumerate(pr):
                            if cls[jt, ic] == MIXED:
                                nc.sync.dma_start(
                                    out=emt[:, j, :],
                                    in_=em.ap()[jt * 128:(jt + 1) * 128, isl])
                            else:
                                nc.vector.memset(emt[:, j, :], 1.0)
                        emop = emt[:, :, 0:qn]
                    ps_s = ps_s_pool.tile([128, 2, 512], F32, name="ps_s")
                    for j, jt in enumerate(pr):
                        nc.tensor.matmul(
                            ps_s[:, j, 0:qn], kT[h][:, jt * 128:(jt + 1) * 128],
                            qT[h][:, ic * 512 + qo:ic * 512 + qo + qn],
                            start=True, stop=True)
                    pt = p_pt.tile([128, 2, 512], F16, name="pt")
                    if len(pr) == 2:
                        nc.scalar.activation(
                            pt[:, :, 0:qn], ps_s[:, :, 0:qn], EXP)
                    else:
                        nc.scalar.activation(
                            pt[:, 0, 0:qn], ps_s[:, 0, 0:qn], EXP)
                    if emop is not None:
                        if len(pr) == 2:
                            nc.vector.tensor_mul(
                                pt[:, :, 0:qn], pt[:, :, 0:qn], emop)
                        else:
                            nc.vector.tensor_mul(
                                pt[:, 0, 0:qn], pt[:, 0, 0:qn], emop[:, 0, :])
                    # DVE row-sum accumulation into the Z pair-planes
                    zsl = slice(qo, qo + qn)
                    if prev is None:
                        if len(pr) == 2:
                            nc.vector.tensor_copy(zt, pt)
                        else:
                            nc.vector.tensor_copy(zt[:, 0, :], pt[:, 0, :])
                            nc.vector.memset(zt[:, 1, :], 0.0)
                    elif len(pr) == 2:
                        with nc.allow_low_precision(reason="softmax denom f16"):
                            nc.vector.tensor_add(
                                zt[:, :, zsl], zt[:, :, zsl], pt[:, :, 0:qn])
                    else:
                        with nc.allow_low_precision(reason="softmax denom f16"):
                            nc.vector.tensor_add(
                                zt[:, 0, zsl], zt[:, 0, zsl], pt[:, 0, 0:qn])
                    if prev is not None:
                        flush(prev)
                        if skip_slots:
                            skip_slots -= 1
                        elif ic_units and inunits < 2:
                            p3_unit(*ic_units.pop(0))
                            inunits += 1
                    prev = (pt, pr, qo, qn)
                flush(prev)
                lps = ps_l_pool.tile([128, 512], F32, name="ps_l", tag="l")
                for j in range(2):
                    nc.tensor.matmul(lps, ones_sb, zt[:, j, :],
                                     start=(j == 0), stop=(j == 1))
                bsb = p_bs.tile([128, 512], F32, name="bsb")
                nc.vector.reciprocal_approx_fast(out=bsb, in_=lps)
                nc.vector.tensor_mul(ctx[h][:, isl], cps, bsb)
                nflush = len(ic_units) if h == HPC - 1 else min(2, len(ic_units))
                for _ in range(nflush):
                    p3_unit(*ic_units.pop(0))
        scope_p2.__exit__(None, None, None)
        scope_p3 = nc.named_scope("P3_out"); scope_p3.__enter__()
        # ---- P3 tail: last token chunk ----------------------------------
        for tt in range(12, 16):
            for nch in range(NI):
                p3_unit(tt, nch, act_ok=True)
        for p in (p_ot, p_z, p_bs, p_em, p_pt):
            p.release()
        for p in (ps_l_pool, ps_c_pool, ps_s_pool):
            p.release()
        pool_qk.release()
        pool_res.release()
        scope_p3.__exit__(None, None, None)

    nc.compile()
    return nc


def _get_nc(cls_key, causal):
    key = (cls_key, causal)
    if key not in _NC_CACHE:
        _NC_CACHE[key] = _build(cls_key, causal)
    return _NC_CACHE[key]


def _causal_pattern(o):
    p = np.arange(128)[:, None]
    f = np.arange(512)[None, :]
    return (p + o * 128 <= f).astype(np.float16)


def kernel(x, Wq, Wk, Wv, Wo, attn_mask):
    x = np.asarray(x, dtype=np.float32)
    Wq = np.asarray(Wq, dtype=np.float32)
    Wk = np.asarray(Wk, dtype=np.float32)
    Wv = np.asarray(Wv, dtype=np.float32)
    Wo = np.asarray(Wo, dtype=np.float32)
    mask = np.asarray(attn_mask, dtype=np.float32).reshape(T, T)

    emT = np.ascontiguousarray(np.exp(mask).T)
    scale = np.float32(1.0 / np.sqrt(DH))

    blocks = emT.reshape(NJ, 128, NI, 512)
    cls = np.full((NJ, NI), MIXED, dtype=np.int64)
    for jt in range(NJ):
        for ic in range(NI):
            sub = blocks[jt, :, ic, :]
            if not sub.any():
                cls[jt, ic] = SKIP
            elif np.all(sub == 1.0):
                cls[jt, ic] = NOMULT
    cls_key = tuple(cls.flatten().tolist())

    # causal fast path: survivors are a prefix, MIXED blocks are the last 4
    # of each i-chunk and match the canonical diagonal patterns
    causal = True
    pat = [_causal_pattern(o).astype(np.float32) for o in range(4)]
    for ic in range(NI):
        surv = [jt for jt in range(NJ) if cls[jt, ic] != SKIP]
        mix = [jt for jt in range(NJ) if cls[jt, ic] == MIXED]
        if surv != list(range(4 * ic + 4)) or mix != list(range(4 * ic, 4 * ic + 4)):
            causal = False
            break
        for jt in mix:
            if not np.array_equal(blocks[jt, :, ic, :], pat[jt - 4 * ic]):
                causal = False
                break
        if not causal:
            break

    em2 = np.zeros((128, 2, 2, 512), dtype=np.float16)
    for kk in range(2):
        for j in range(2):
            em2[:, kk, j, :] = _causal_pattern(2 * kk + j)

    def _perm(a, ntile):
        # [ntile*128, F] -> [128, ntile, F] contiguous fp16
        f = a.shape[1]
        return np.ascontiguousarray(
            a.reshape(ntile, 128, f).transpose(1, 0, 2)).astype(np.float16)

    xT = [_perm(x[b].T, ND) for b in range(B)]
    emT16 = emT.astype(np.float16)

    in_maps = []
    for c in range(8):
        b, g = c // 4, c % 4
        rows = slice(E * g, E * (g + 1))
        m = {
            "xt": xT[b],
            "wq": _perm((Wq[rows, :] * scale).T, ND),
            "wk": _perm(Wk[rows, :].T, ND),
            "wv": _perm(Wv[rows, :].T, ND),
            "wo": _perm(Wo[:, rows].T, NE),
            "ons": np.ones((128, 128), dtype=np.float16),
        }
        if causal:
            m["em2"] = em2
        else:
            m["em"] = emT16
        in_maps.append(m)

    global _LAST_IN_MAPS, _LAST_NC
    _LAST_IN_MAPS = in_maps
    nc = _get_nc(cls_key, causal)
    _LAST_NC = nc
    res = run_bass_kernel_spmd(nc, in_maps, list(range(8)))
    outs = [r["out"].astype(np.float32) for r in res.results]
    full = np.stack([
        outs[0] + outs[1] + outs[2] + outs[3],
        outs[4] + outs[5] + outs[6] + outs[7],
    ]).astype(np.float32)
    return full


# revision 47
# speedup vs baseline: 1.0153x; 1.0101x over previous
"""Fused multi-head attention (B=2, T=2048, D=2048, H=16) on 8 trn2 NeuronCores.

Sharding: core c handles batch b=c//4 and heads [4g, 4g+4), g=c%4 (tensor
parallel over heads x data parallel over batch). Each core computes its
4 heads' contribution to out[b] = attn(x[b]) @ Wo^T; the host sums the 4
partials per batch.

v2: all matmul operands fp16 (PSUM stays fp32), x^T DMA'd once into
resident SBUF, weights prefetched at kernel start, v computed directly in
[token, feature] layout (x-stationary matmuls, no PE transposes), exp
batched over j-tile pairs ([128,1024] ACT calls), causal diagonal masks as
2 resident [128,2,512] pattern tiles, software-pipelined attention inner
loop sized to exactly 8 PSUM banks, fp16 output.

Device algorithm (per core, E=512 features = 4 heads):
  P1  qT/kT = (W_s) @ x^T   [E rows as 4x(dh=128), T]   (Wq pre-scaled)
      v     = x @ Wv_s^T    [T, E]
  P2  per i-chunk (512 q), head pair: S^T pair = kT_jt^T-contract @ qT
        -> exp (ACT, [128,1024]) -> *mask (diag pairs) ->
        ctx^T += v_jt^T @ P^T ; L += ones128^T @ P^T (l replicated on all
        128 partitions, same PE tile config as ctx, no broadcast needed);
        ctx^T *= recip(L).  Upper-diagonal pairs trimmed to 256 queries.
  P3  out[t, d] = sum_e ctx^T[e, t] * WoT[e, d] -> DRAM (fp16)
"""

import numpy as np

import concourse.bass as bass
import concourse.mybir as mybir
import concourse.tile as tile
from concourse import bacc
from concourse.bass_utils import run_bass_kernel_spmd

F32 = mybir.dt.float32
F16 = mybir.dt.float16
EXP = mybir.ActivationFunctionType.Exp

B, T, D, H = 2, 2048, 2048, 16
DH = D // H          # 128
E = 512              # features per core (4 heads)
HPC = 4              # heads per core
NT = T // 128        # 16 token tiles
ND = D // 128        # 16 model-dim tiles
NE = E // 128        # 4 e-tiles per core
NI = T // 512        # 4 i-chunks (query chunks)
NJ = NT              # 16 j-tiles (key tiles)
NCH = T // 1024      # 2 big token chunks for the projections

_NC_CACHE = {}

# per-(jt, ic) mask-block class: 0 = fully masked (skip), 1 = unmasked
# (skip the mask multiply), 2 = mixed (multiply by exp(mask) elementwise)
SKIP, NOMULT, MIXED = 0, 1, 2


def _build(cls_key, causal):
    cls = np.asarray(cls_key, dtype=np.int64).reshape(NJ, NI)
    nc = bacc.Bacc(None, target_bir_lowering=False, debug=False)
    # all staged operands are host-permuted to SBUF layout [128, tile, free]
    # so each one is a single large DMA
    xt = nc.declare_dram_parameter("xt", [128, ND, T], F16, isOutput=False)
    wq = nc.declare_dram_parameter("wq", [128, ND, E], F16, isOutput=False)
    wk = nc.declare_dram_parameter("wk", [128, ND, E], F16, isOutput=False)
    wv = nc.declare_dram_parameter("wv", [128, ND, E], F16, isOutput=False)
    wo = nc.declare_dram_parameter("wo", [128, NE, D], F16, isOutput=False)
    if causal:
        em2 = nc.declare_dram_parameter("em2", [128, 2, 2, 512], F16, isOutput=False)
    else:
        em = nc.declare_dram_parameter("em", [T, T], F16, isOutput=False)
    ons = nc.declare_dram_parameter("ons", [128, 128], F16, isOutput=False)
    out = nc.declare_dram_parameter("out", [T, D], F16, isOutput=True)

    with tile.TileContext(nc) as tc:
        # ---- long-lived residents --------------------------------------
        pool_res = tc.alloc_tile_pool(name="res", bufs=1)
        ctx = [pool_res.tile([128, T], F16, name=f"ctx{m}") for m in range(NE)]
        v_sb = pool_res.tile([128, NT, E], F16)
        wo_sb = pool_res.tile([128, NE, D], F16)
        ones_sb = pool_res.tile([128, 128], F16)
        scratch = pool_res.tile([1, 8], F16)
        if causal:
            em_sb = pool_res.tile([128, 2, 2, 512], F16)

        pool_qk = tc.alloc_tile_pool(name="res_qk", bufs=1)
        qT = [pool_qk.tile([128, T], F16, name=f"qT{m}") for m in range(NE)]
        kT = [pool_qk.tile([128, T], F16, name=f"kT{m}") for m in range(NE)]

        # One set of PSUM pools for every phase: P1's v-pass borrows the
        # "l" banks and q/k the "c" banks, so there is no PSUM pool
        # transition (and no pool-release barrier) at the P1->P2 seam.
        ps_s_pool = tc.alloc_tile_pool(name="pss", bufs=2, space="PSUM")
        ps_c_pool = tc.alloc_tile_pool(name="psc", bufs=3, space="PSUM")
        ps_l_pool = tc.alloc_tile_pool(name="psl", bufs=1, space="PSUM")

        pool_p1 = tc.alloc_tile_pool(name="p1", bufs=1)
        xt_sb = pool_p1.tile([128, ND, T], F16)
        wq_sb = pool_p1.tile([128, ND, E], F16)
        wk_sb = pool_p1.tile([128, ND, E], F16)
        wv_sb = pool_p1.tile([128, ND, E], F16)

        # ---- DMA schedule: wv + per-token-block x first so the v-pass can
        # start after ~2.5MB instead of gating on the full 6MB q-projection
        # working set; the rest streams underneath the v-pass compute.
        nc.sync.dma_start(out=wv_sb[:, 0:4, :], in_=wv.ap()[:, 0:4, :])
        nc.sync.dma_start(out=xt_sb[:, :, 0:128], in_=xt.ap()[:, :, 0:128])
        for q in range(1, 4):
            nc.sync.dma_start(out=wv_sb[:, 4 * q:4 * q + 4, :],
                              in_=wv.ap()[:, 4 * q:4 * q + 4, :])
        for tb in range(1, 8):
            tsl = slice(tb * 128, (tb + 1) * 128)
            nc.sync.dma_start(out=xt_sb[:, :, tsl], in_=xt.ap()[:, :, tsl])
        nc.sync.dma_start(out=xt_sb[:, :, 1024:1536], in_=xt.ap()[:, :, 1024:1536])
        nc.sync.dma_start(out=xt_sb[:, :, 1536:2048], in_=xt.ap()[:, :, 1536:2048])
        nc.sync.dma_start(out=wq_sb, in_=wq.ap())
        nc.sync.dma_start(out=wk_sb, in_=wk.ap())
        nc.sync.dma_start(out=ones_sb, in_=ons.ap())
        if causal:
            nc.sync.dma_start(out=em_sb, in_=em2.ap())
        nc.sync.dma_start(out=wo_sb, in_=wo.ap())

        # warm the ACT exp table set before P2 needs it
        nc.scalar.activation(scratch[0:1, 0:1], wv_sb[0:1, 0, 0:1], EXP)

        scope_p1 = nc.named_scope("P1_qkv"); scope_p1.__enter__()
        # ---- P1: v (token-major) first, then q/k (feature-major) --------
        for tbg in range(NT):
            ps = ps_c_pool.tile([128, 512], F32, name="ps_v", tag="c")
            for dt in range(ND):
                nc.tensor.matmul(
                    ps, xt_sb[:, dt, tbg * 128:(tbg + 1) * 128],
                    wv_sb[:, dt, :], start=(dt == 0), stop=(dt == ND - 1))
            if tbg % 2 == 0:
                nc.vector.tensor_copy(v_sb[:, tbg, :], ps)
            else:
                nc.scalar.copy(v_sb[:, tbg, :], ps)
        for tch in range(NCH):
            for ti, (w_sb, dst) in enumerate(((wq_sb, qT), (wk_sb, kT))):
                for m in range(NE):
                    for half in range(2):
                        tsl = slice(tch * 1024 + half * 512,
                                    tch * 1024 + (half + 1) * 512)
                        ps = ps_c_pool.tile([128, 512], F32, name="ps_qk", tag="c")
                        for dt in range(ND):
                            nc.tensor.matmul(ps, w_sb[:, dt, m * 128:(m + 1) * 128],
                                             xt_sb[:, dt, tsl],
                                             start=(dt == 0), stop=(dt == ND - 1))
                        if (m + ti + half) % 2 == 0:
                            nc.scalar.copy(dst[m][:, tsl], ps)
                        else:
                            nc.vector.tensor_copy(dst[m][:, tsl], ps)
        pool_p1.release()
        scope_p1.__exit__(None, None, None)

        scope_p2 = nc.named_scope("P2_attn"); scope_p2.__enter__()
        # ---- P2: attention + fused output projection --------------------
        # The softmax row-sums are accumulated on the (otherwise idle) DVE
        # into a Z pair-plane tile, leaving the PE stream S+ctx only; the
        # freed PE slack is filled with P3 output-projection units emitted
        # at each (chunk, head) epilogue (chunk ic-1, 4 units per head).
        p_pt = tc.alloc_tile_pool(name="p2pt", bufs=6)
        p_em = tc.alloc_tile_pool(name="p2em", bufs=3)
        p_bs = tc.alloc_tile_pool(name="p2bs", bufs=3)
        p_z = tc.alloc_tile_pool(name="p2z", bufs=3)
        p_ot = tc.alloc_tile_pool(name="p2ot", bufs=6)

        def p3_unit(tt, nch, act_ok=False):
            tsl = slice(tt * 128, (tt + 1) * 128)
            ps_o = ps_c_pool.tile([128, 512], F32, name="ps_o", tag="c")
            for et in range(NE):
                nc.tensor.matmul(
                    ps_o, ctx[et][:, tsl],
                    wo_sb[:, et, nch * 512:(nch + 1) * 512],
                    start=(et == 0), stop=(et == NE - 1))
            ot = p_ot.tile([128, 512], F16, name="ot")
            # inside the attention window ACT is reserved for the exps (an
            # interleaved copy there delays the exp chain and stalls ctx)
            if act_ok and (tt + nch) % 2 == 0:
                nc.scalar.copy(ot, ps_o)
            else:
                nc.vector.tensor_copy(ot, ps_o)
            nc.sync.dma_start(
                out=out.ap()[tsl, nch * 512:(nch + 1) * 512], in_=ot)

        for ic in range(NI):
            isl = slice(ic * 512, (ic + 1) * 512)
            surv = [jt for jt in range(NJ) if cls[jt, ic] != SKIP]
            assert surv, f"i-chunk {ic}: every key block masked"
            first, last = surv[0], surv[-1]
            pairs = [tuple(surv[i:i + 2]) for i in range(0, len(surv), 2)]
            # P3 units for the previous chunk, shared across this chunk's
            # four head loops to fill the PE's exp-wait slack
            ic_units = [(4 * (ic - 1) + tt, u)
                        for tt in range(4) for u in range(4)] if ic else []
            for h in range(HPC):
                cps = ps_c_pool.tile([128, 512], F32, name="ps_c", tag="c")
                zt = p_z.tile([128, 2, 512], F16, name="zt")
                # h0's first slot stays empty: the previous chunk's last
                # ctx normalization is still draining on the DVE
                skip_slots = 1 if h == 0 else 0
                inunits = 0

                def flush(prev):
                    ppt, ppr, pqo, pqn = prev
                    psl = slice(pqo, pqo + pqn)
                    for j, jt in enumerate(ppr):
                        st, sp = jt == first, jt == last
                        nc.tensor.matmul(
                            cps[:, psl], v_sb[:, jt, h * 128:(h + 1) * 128],
                            ppt[:, j, 0:pqn], start=st, stop=sp,
                            skip_group_check=True)

                prev = None
                for pr in pairs:
                    # query-range trim: the upper diagonal pair only attends
                    # to the last 256 queries of the chunk
                    if causal and cls[pr[0], ic] == MIXED and pr[0] == 4 * ic + 2:
                        qo, qn = 256, 256
                    else:
                        qo, qn = 0, 512
                    # mask operand (None / resident slice / DMA'd)
                    emop = None
                    if causal and cls[pr[0], ic] == MIXED:
                        # both diagonal pairs reduce to the (o0, o1) patterns
                        # over their query window
                        emop = em_sb[:, 0, :, 0:qn]
                    elif not causal and any(cls[jt, ic] == MIXED for jt in pr):
                        emt = p_em.tile([128, 2, 512], F16, name="emt")
                        for j, jt in enumerate(pr):
                            if cls[jt, ic] == MIXED:
                                nc.sync.dma_start(
                                    out=emt[:, j, :],
                                    in_=em.ap()[jt * 128:(jt + 1) * 128, isl])
                            else:
                                nc.vector.memset(emt[:, j, :], 1.0)
                        emop = emt[:, :, 0:qn]
                    ps_s = ps_s_pool.tile([128, 2, 512], F32, name="ps_s")
                    for j, jt in enumerate(pr):
                        nc.tensor.matmul(
                            ps_s[:, j, 0:qn], kT[h][:, jt * 128:(jt + 1) * 128],
                            qT[h][:, ic * 512 + qo:ic * 512 + qo + qn],
                            start=True, stop=True)
                    pt = p_pt.tile([128, 2, 512], F16, name="pt")
                    if len(pr) == 2:
                        nc.scalar.activation(
                            pt[:, :, 0:qn], ps_s[:, :, 0:qn], EXP)
                    else:
                        nc.scalar.activation(
                            pt[:, 0, 0:qn], ps_s[:, 0, 0:qn], EXP)
                    if emop is not None:
                        if len(pr) == 2:
                            nc.vector.tensor_mul(
                                pt[:, :, 0:qn], pt[:, :, 0:qn], emop)
                        else:
                            nc.vector.tensor_mul(
                                pt[:, 0, 0:qn], pt[:, 0, 0:qn], emop[:, 0, :])
                    # DVE row-sum accumulation into the Z pair-planes
                    zsl = slice(qo, qo + qn)
                    if prev is None:
                        if len(pr) == 2:
                            nc.vector.tensor_copy(zt, pt)
                        else:
                            nc.vector.tensor_copy(zt[:, 0, :], pt[:, 0, :])
                            nc.vector.memset(zt[:, 1, :], 0.0)
                    elif len(pr) == 2:
                        with nc.allow_low_precision(reason="softmax denom f16"):
                            nc.vector.tensor_add(
                                zt[:, :, zsl], zt[:, :, zsl], pt[:, :, 0:qn])
                    else:
                        with nc.allow_low_precision(reason="softmax denom f16"):
                            nc.vector.tensor_add(
                                zt[:, 0, zsl], zt[:, 0, zsl], pt[:, 0, 0:qn])
                    if prev is not None:
                        # fill unit BEFORE the ctx flush: covers the first
                        # exp's latency at each head start
                        if skip_slots:
                            skip_slots -= 1
                        elif ic_units and inunits < 2:
                            p3_unit(*ic_units.pop(0))
                            inunits += 1
                        flush(prev)
                    prev = (pt, pr, qo, qn)
                flush(prev)
                lps = ps_l_pool.tile([128, 512], F32, name="ps_l", tag="l")
                for j in range(2):
                    nc.tensor.matmul(lps, ones_sb, zt[:, j, :],
                                     start=(j == 0), stop=(j == 1))
                bsb = p_bs.tile([128, 512], F32, name="bsb")
                nc.vector.reciprocal_approx_fast(out=bsb, in_=lps)
                nc.vector.tensor_mul(ctx[h][:, isl], cps, bsb)
                nflush = len(ic_units) if h == HPC - 1 else min(2, len(ic_units))
                for _ in range(nflush):
                    p3_unit(*ic_units.pop(0))
        scope_p2.__exit__(None, None, None)
        scope_p3 = nc.named_scope("P3_out"); scope_p3.__enter__()
        # ---- P3 tail: last token chunk ----------------------------------
        for tt in range(12, 16):
            for nch in range(NI):
                p3_unit(tt, nch, act_ok=True)
        for p in (p_ot, p_z, p_bs, p_em, p_pt):
            p.release()
        for p in (ps_l_pool, ps_c_pool, ps_s_pool):
            p.release()
        pool_qk.release()
        pool_res.release()
        scope_p3.__exit__(None, None, None)

    nc.compile()
    return nc


def _get_nc(cls_key, causal):
    key = (cls_key, causal)
    if key not in _NC_CACHE:
        _NC_CACHE[key] = _build(cls_key, causal)
    return _NC_CACHE[key]


def _causal_pattern(o):
    p = np.arange(128)[:, None]
    f = np.arange(512)[None, :]
    return (p + o * 128 <= f).astype(np.float16)


def kernel(x, Wq, Wk, Wv, Wo, attn_mask):
    x = np.asarray(x, dtype=np.float32)
    Wq = np.asarray(Wq, dtype=np.float32)
    Wk = np.asarray(Wk, dtype=np.float32)
    Wv = np.asarray(Wv, dtype=np.float32)
    Wo = np.asarray(Wo, dtype=np.float32)
    mask = np.asarray(attn_mask, dtype=np.float32).reshape(T, T)

    emT = np.ascontiguousarray(np.exp(mask).T)
    scale = np.float32(1.0 / np.sqrt(DH))

    blocks = emT.reshape(NJ, 128, NI, 512)
    cls = np.full((NJ, NI), MIXED, dtype=np.int64)
    for jt in range(NJ):
        for ic in range(NI):
            sub = blocks[jt, :, ic, :]
            if not sub.any():
                cls[jt, ic] = SKIP
            elif np.all(sub == 1.0):
                cls[jt, ic] = NOMULT
    cls_key = tuple(cls.flatten().tolist())

    # causal fast path: survivors are a prefix, MIXED blocks are the last 4
    # of each i-chunk and match the canonical diagonal patterns
    causal = True
    pat = [_causal_pattern(o).astype(np.float32) for o in range(4)]
    for ic in range(NI):
        surv = [jt for jt in range(NJ) if cls[jt, ic] != SKIP]
        mix = [jt for jt in range(NJ) if cls[jt, ic] == MIXED]
        if surv != list(range(4 * ic + 4)) or mix != list(range(4 * ic, 4 * ic + 4)):
            causal = False
            break
        for jt in mix:
            if not np.array_equal(blocks[jt, :, ic, :], pat[jt - 4 * ic]):
                causal = False
                break
        if not causal:
            break

    em2 = np.zeros((128, 2, 2, 512), dtype=np.float16)
    for kk in range(2):
        for j in range(2):
            em2[:, kk, j, :] = _causal_pattern(2 * kk + j)

    def _perm(a, ntile):
        # [ntile*128, F] -> [128, ntile, F] contiguous fp16
        f = a.shape[1]
        return np.ascontiguousarray(
            a.reshape(ntile, 128, f).transpose(1, 0, 2)).astype(np.float16)

    xT = [_perm(x[b].T, ND) for b in range(B)]
    emT16 = emT.astype(np.float16)

    in_maps = []
    for c in range(8):
        b, g = c // 4, c % 4
        rows = slice(E * g, E * (g + 1))
        m = {
            "xt": xT[b],
            "wq": _perm((Wq[rows, :] * scale).T, ND),
            "wk": _perm(Wk[rows, :].T, ND),
            "wv": _perm(Wv[rows, :].T, ND),
            "wo": _perm(Wo[:, rows].T, NE),
            "ons": np.ones((128, 128), dtype=np.float16),
        }
        if causal:
            m["em2"] = em2
        else:
            m["em"] = emT16
        in_maps.append(m)

    global _LAST_IN_MAPS, _LAST_NC
    _LAST_IN_MAPS = in_maps
    nc = _get_nc(cls_key, causal)
    _LAST_NC = nc
    res = run_bass_kernel_spmd(nc, in_maps, list(range(8)))
    outs = [r["out"].astype(np.float32) for r in res.results]
    full = np.stack([
        outs[0] + outs[1] + outs[2] + outs[3],
        outs[4] + outs[5] + outs[6] + outs[7],
    ]).astype(np.float32)
    return full
